# revision 3
# baseline (speedup 1.0000x reference)
"""MoE expert-FFN (nn_Experts) Trainium2 kernel.

Expert-parallel: one expert per NeuronCore (E = 8 = n_cores).
Host does the token gather (dispatch) and weighted scatter-add (combine);
each core runs the fused FFN for its expert:

    hT = gelu_tanh(W1^T @ tokT + b1)        # [F, C] on chip, f32r matmuls
    out = (hT^T @ W2) * w[:, None]          # [C, D], combine weight fused
                                            # into the PSUM eviction

Layouts are pre-packed on host so every DMA is contiguous-row strided:
    tokT [P, D/P, C]   (d = kc*P + p)
    W1   [P, D/P, F]   (d = kc*P + p)
    W2   [P, F/P, D]   (f = kc*P + p)
    b1t  [P, F/P]      (f = fb*P + p)
    wv   [P, C/CB, CB/P]  (c = cb*CB + m*P + p)
"""
import numpy as np

import concourse.bacc as bacc
import concourse.tile as tile
from concourse import mybir
from concourse.bass_utils import run_bass_kernel_spmd

P = 128
T, D, F, E, C = 8192, 2048, 8192, 8, 2048
CB = 512          # capacity block held resident as hT [F, CB]
NOUT = 512        # output free-dim tile (one PSUM bank of fp32)

f32 = mybir.dt.float32
f32r = mybir.dt.float32r
AF = mybir.ActivationFunctionType


def build_nc(d=None, f=None, c=None, cb=None, nout=None):
    d = D if d is None else d
    f = F if f is None else f
    c = C if c is None else c
    cb = CB if cb is None else cb
    nout = NOUT if nout is None else nout
    KD = d // P       # mm1 contraction chunks
    KF = f // P       # mm2 contraction chunks
    FB = f // P       # mm1 output partition groups
    MB = cb // P      # output row subtiles per c-block
    NB = d // nout    # output col tiles
    NCB = c // cb     # c blocks

    nc = bacc.Bacc()
    tokT = nc.declare_dram_parameter("tokT", [P, KD, c], f32r, isOutput=False)
    w1 = nc.declare_dram_parameter("w1", [P, KD, f], f32r, isOutput=False)
    w2 = nc.declare_dram_parameter("w2", [P, KF, d], f32r, isOutput=False)
    b1t = nc.declare_dram_parameter("b1t", [P, FB], f32, isOutput=False)
    wv = nc.declare_dram_parameter("wv", [P, NCB, MB], f32, isOutput=False)
    out = nc.declare_dram_parameter("out", [c, d], f32, isOutput=True)

    with tile.TileContext(nc) as tc:
        with tc.tile_pool(name="const", bufs=1) as const, \
             tc.tile_pool(name="tokp", bufs=1) as tokp, \
             tc.tile_pool(name="hp", bufs=1) as hp, \
             tc.tile_pool(name="w1p", bufs=2) as w1p, \
             tc.tile_pool(name="w2p", bufs=4) as w2p, \
             tc.tile_pool(name="ostp", bufs=3) as ostp, \
             tc.tile_pool(name="php", bufs=2, space="PSUM") as php, \
             tc.tile_pool(name="pop", bufs=1, space="PSUM") as pop:
            b1s = const.tile([P, FB], f32)
            nc.sync.dma_start(b1s[:], b1t[:])
            wvs = const.tile([P, NCB, MB], f32)
            nc.sync.dma_start(wvs[:], wv[:])

            for cbi in range(NCB):
                tok_c = tokp.tile([P, KD, cb], f32r, tag="tok")
                nc.sync.dma_start(tok_c[:], tokT[:, :, cbi * cb:(cbi + 1) * cb])
                hT = hp.tile([P, KF, cb], f32r, tag="hT")

                # mm1: hT[f, :] = gelu(W1^T @ tokT + b1), one 128-row f group
                # per PSUM accumulation
                for fb in range(FB):
                    w1t = w1p.tile([P, KD, P], f32r, tag="w1t")
                    nc.sync.dma_start(w1t[:], w1[:, :, fb * P:(fb + 1) * P])
                    ph = php.tile([P, cb], f32, tag="ph")
                    for kc in range(KD):
                        nc.tensor.matmul(ph[:], w1t[:, kc, :], tok_c[:, kc, :],
                                         start=(kc == 0), stop=(kc == KD - 1))
                    nc.scalar.activation(hT[:, fb, :], ph[:], AF.Gelu_apprx_tanh,
                                         bias=b1s[:, fb:fb + 1])

                # mm2: out[c, :] = (hT^T @ W2) * w, accumulated over all of F
                for nb in range(NB):
                    pos = [pop.tile([P, nout], f32, tag=f"po{m}", name=f"po{m}")
                           for m in range(MB)]
                    for kc in range(KF):
                        w2t = w2p.tile([P, nout], f32r, tag="w2t")
                        nc.sync.dma_start(
                            w2t[:], w2[:, kc, nb * nout:(nb + 1) * nout])
                        for m in range(MB):
                            nc.tensor.matmul(pos[m][:],
                                             hT[:, kc, m * P:(m + 1) * P],
                                             w2t[:],
                                             start=(kc == 0), stop=(kc == KF - 1))
                    for m in range(MB):
                        ost = ostp.tile([P, nout], f32, tag="ost")
                        nc.scalar.activation(ost[:], pos[m][:], AF.Copy,
                                             scale=wvs[:, cbi, m:m + 1])
                        r0 = cbi * cb + m * P
                        nc.sync.dma_start(
                            out[r0:r0 + P, nb * nout:(nb + 1) * nout], ost[:])
    nc.compile()
    return nc


def pack_core(inputs, inputs_weight, top_idx, W1, b1, e,
              d=None, f=None, c=None, cb=None):
    d = D if d is None else d
    f = F if f is None else f
    c = C if c is None else c
    cb = CB if cb is None else cb
    """Host-side dispatch: gather + relayout for expert e."""
    KD = d // P
    FB = f // P
    MB = cb // P
    NCB = c // cb
    idx = np.asarray(top_idx[:, e])
    tok = np.ascontiguousarray(inputs[idx])                      # [c, d]
    tokT = np.ascontiguousarray(tok.T).reshape(KD, P, c).transpose(1, 0, 2)
    w1m = np.ascontiguousarray(W1[e]).reshape(KD, P, f).transpose(1, 0, 2)
    b1m = np.ascontiguousarray(b1[e]).reshape(FB, P).T
    wvm = np.ascontiguousarray(inputs_weight[idx, e]) \
        .reshape(NCB, MB, P).transpose(2, 0, 1)
    return idx, tok, tokT, w1m, b1m, wvm


_NC_CACHE = {}


def get_nc():
    key = (D, F, C, CB, NOUT)
    if key not in _NC_CACHE:
        _NC_CACHE[key] = build_nc()
    return _NC_CACHE[key]


def make_in_maps(inputs, inputs_weight, top_idx, W1, b1, W2, b2):
    KF = F // P
    in_maps = []
    idxs = []
    for e in range(E):
        idx, _tok, tokT, w1m, b1m, wvm = pack_core(
            inputs, inputs_weight, top_idx, W1, b1, e)
        w2m = np.ascontiguousarray(W2[e]).reshape(KF, P, D).transpose(1, 0, 2)
        in_maps.append({
            "tokT": np.ascontiguousarray(tokT, dtype=np.float32),
            "w1": np.ascontiguousarray(w1m, dtype=np.float32),
            "w2": np.ascontiguousarray(w2m, dtype=np.float32),
            "b1t": np.ascontiguousarray(b1m, dtype=np.float32),
            "wv": np.ascontiguousarray(wvm, dtype=np.float32),
        })
        idxs.append(idx)
    return in_maps, idxs


def combine(outs, idxs, inputs_weight, top_idx, b2):
    """Host-side combine: weighted scatter-add back to token positions."""
    vals = []
    for e in range(E):
        v = outs[e]
        if np.any(b2[e]):
            w_e = inputs_weight[idxs[e], e].astype(np.float32)
            v = v + w_e[:, None] * b2[e][None, :].astype(np.float32)
        vals.append(v)
    vals = np.concatenate(vals, axis=0)          # [E*C, D]
    idx_all = np.concatenate(idxs, axis=0)       # [E*C]

    order = np.argsort(idx_all, kind="stable")
    si = idx_all[order]
    sv = vals[order]
    starts = np.flatnonzero(np.r_[True, si[1:] != si[:-1]])
    sums = np.add.reduceat(sv, starts, axis=0)
    res = np.zeros((T, D), dtype=np.float32)
    res[si[starts]] = sums
    return res


def kernel(inputs, inputs_weight, top_idx, W1, b1, W2, b2):
    inputs = np.asarray(inputs, dtype=np.float32)
    inputs_weight = np.asarray(inputs_weight, dtype=np.float32)
    top_idx = np.asarray(top_idx)
    W1 = np.asarray(W1, dtype=np.float32)
    b1 = np.asarray(b1, dtype=np.float32)
    W2 = np.asarray(W2, dtype=np.float32)
    b2 = np.asarray(b2, dtype=np.float32)

    nc = get_nc()
    in_maps, idxs = make_in_maps(
        inputs, inputs_weight, top_idx, W1, b1, W2, b2)
    r = run_bass_kernel_spmd(nc, in_maps, list(range(E)))
    outs = [r.results[e]["out"] for e in range(E)]
    return combine(outs, idxs, inputs_weight, top_idx, b2)


# revision 6
# speedup vs baseline: 4.0310x; 4.0310x over previous
"""MoE expert-FFN (nn_Experts) Trainium2 kernel.

Expert-parallel: one expert per NeuronCore (E = 8 = n_cores).
Host does the token gather (dispatch) and weighted scatter-add (combine);
each core runs the fused FFN for its expert:

    hT = gelu_tanh(W1^T @ tokT + b1)        # [F, C] on chip, f32r matmuls
    out = (hT^T @ W2) * w[:, None]          # [C, D], combine weight fused
                                            # into the PSUM eviction

Layouts are pre-packed on host so every DMA is contiguous-row strided:
    tokT [P, D/P, C]   (d = kc*P + p)
    W1   [P, D/P, F]   (d = kc*P + p)
    W2   [P, F/P, D]   (f = kc*P + p)
    b1t  [P, F/P]      (f = fb*P + p)
    wv   [P, C/CB, CB/P]  (c = cb*CB + m*P + p)
"""
import numpy as np

import concourse.bacc as bacc
import concourse.tile as tile
from concourse import mybir
from concourse.bass_utils import run_bass_kernel_spmd

P = 128
T, D, F, E, C = 8192, 2048, 8192, 8, 2048
CB = 512          # capacity block held resident as hT [F, CB]
NOUT = 512        # output free-dim tile (one PSUM bank of fp32)

f32 = mybir.dt.float32
f32r = mybir.dt.float32r
AF = mybir.ActivationFunctionType


def build_nc(d=None, f=None, c=None, cb=None, nout=None):
    d = D if d is None else d
    f = F if f is None else f
    c = C if c is None else c
    cb = CB if cb is None else cb
    nout = NOUT if nout is None else nout
    KD = d // P       # mm1 contraction chunks
    KF = f // P       # mm2 contraction chunks
    FB = f // P       # mm1 output partition groups
    MB = cb // P      # output row subtiles per c-block
    NB = d // nout    # output col tiles
    NCB = c // cb     # c blocks

    nc = bacc.Bacc()
    tokT = nc.declare_dram_parameter("tokT", [P, KD, c], f32r, isOutput=False)
    w1 = nc.declare_dram_parameter("w1", [P, KD, f], f32r, isOutput=False)
    w2 = nc.declare_dram_parameter("w2", [P, KF, d], f32r, isOutput=False)
    b1t = nc.declare_dram_parameter("b1t", [P, FB], f32, isOutput=False)
    wv = nc.declare_dram_parameter("wv", [P, NCB, MB], f32, isOutput=False)
    out = nc.declare_dram_parameter("out", [c, d], f32, isOutput=True)

    with tile.TileContext(nc) as tc:
        with tc.tile_pool(name="const", bufs=1) as const, \
             tc.tile_pool(name="tokp", bufs=1) as tokp, \
             tc.tile_pool(name="hp", bufs=1) as hp, \
             tc.tile_pool(name="w1p", bufs=2) as w1p, \
             tc.tile_pool(name="w2p", bufs=4) as w2p, \
             tc.tile_pool(name="ostp", bufs=3) as ostp, \
             tc.tile_pool(name="php", bufs=3, space="PSUM") as php, \
             tc.tile_pool(name="pop", bufs=1, space="PSUM") as pop:
            b1s = const.tile([P, FB], f32)
            nc.sync.dma_start(b1s[:], b1t[:])
            wvs = const.tile([P, NCB, MB], f32)
            nc.sync.dma_start(wvs[:], wv[:])

            for cbi in range(NCB):
                tok_c = tokp.tile([P, KD, cb], f32r, tag="tok")
                nc.sync.dma_start(tok_c[:], tokT[:, :, cbi * cb:(cbi + 1) * cb])
                hT = hp.tile([P, KF, cb], f32r, tag="hT")

                # mm1: hT[f, :] = gelu(W1^T @ tokT + b1), one 128-row f group
                # per PSUM accumulation
                for fb in range(FB):
                    w1t = w1p.tile([P, KD, P], f32r, tag="w1t")
                    nc.sync.dma_start(w1t[:], w1[:, :, fb * P:(fb + 1) * P])
                    ph = php.tile([P, cb], f32, tag="ph")
                    for kc in range(KD):
                        nc.tensor.matmul(ph[:], w1t[:, kc, :], tok_c[:, kc, :],
                                         start=(kc == 0), stop=(kc == KD - 1))
                    # fast DVE drain of PSUM, then gelu in place on ACT off
                    # the PE critical path
                    nc.vector.tensor_copy(hT[:, fb, :], ph[:])
                    nc.scalar.activation(hT[:, fb, :], hT[:, fb, :],
                                         AF.Gelu_apprx_tanh,
                                         bias=b1s[:, fb:fb + 1])

                # mm2: out[c, :] = (hT^T @ W2) * w, accumulated over all of F
                for nb in range(NB):
                    pos = [pop.tile([P, nout], f32, tag=f"po{m}", name=f"po{m}")
                           for m in range(MB)]
                    for kc in range(KF):
                        w2t = w2p.tile([P, nout], f32r, tag="w2t")
                        nc.sync.dma_start(
                            w2t[:], w2[:, kc, nb * nout:(nb + 1) * nout])
                        for m in range(MB):
                            nc.tensor.matmul(pos[m][:],
                                             hT[:, kc, m * P:(m + 1) * P],
                                             w2t[:],
                                             start=(kc == 0), stop=(kc == KF - 1))
                    for m in range(MB):
                        ost = ostp.tile([P, nout], f32, tag="ost")
                        nc.vector.tensor_tensor(
                            ost[:], pos[m][:],
                            wvs[:, cbi, m:m + 1].to_broadcast((P, nout)),
                            mybir.AluOpType.mult)
                        r0 = cbi * cb + m * P
                        nc.sync.dma_start(
                            out[r0:r0 + P, nb * nout:(nb + 1) * nout], ost[:])
    nc.compile()
    return nc


def pack_core(inputs, inputs_weight, top_idx, W1, b1, e,
              d=None, f=None, c=None, cb=None):
    d = D if d is None else d
    f = F if f is None else f
    c = C if c is None else c
    cb = CB if cb is None else cb
    """Host-side dispatch: gather + relayout for expert e."""
    KD = d // P
    FB = f // P
    MB = cb // P
    NCB = c // cb
    idx = np.asarray(top_idx[:, e])
    tok = np.ascontiguousarray(inputs[idx])                      # [c, d]
    tokT = np.ascontiguousarray(tok.T).reshape(KD, P, c).transpose(1, 0, 2)
    w1m = np.ascontiguousarray(W1[e]).reshape(KD, P, f).transpose(1, 0, 2)
    b1m = np.ascontiguousarray(b1[e]).reshape(FB, P).T
    wvm = np.ascontiguousarray(inputs_weight[idx, e]) \
        .reshape(NCB, MB, P).transpose(2, 0, 1)
    return idx, tok, tokT, w1m, b1m, wvm


_NC_CACHE = {}


def get_nc():
    key = (D, F, C, CB, NOUT)
    if key not in _NC_CACHE:
        _NC_CACHE[key] = build_nc()
    return _NC_CACHE[key]


def make_in_maps(inputs, inputs_weight, top_idx, W1, b1, W2, b2):
    KF = F // P
    in_maps = []
    idxs = []
    for e in range(E):
        idx, _tok, tokT, w1m, b1m, wvm = pack_core(
            inputs, inputs_weight, top_idx, W1, b1, e)
        w2m = np.ascontiguousarray(W2[e]).reshape(KF, P, D).transpose(1, 0, 2)
        in_maps.append({
            "tokT": np.ascontiguousarray(tokT, dtype=np.float32),
            "w1": np.ascontiguousarray(w1m, dtype=np.float32),
            "w2": np.ascontiguousarray(w2m, dtype=np.float32),
            "b1t": np.ascontiguousarray(b1m, dtype=np.float32),
            "wv": np.ascontiguousarray(wvm, dtype=np.float32),
        })
        idxs.append(idx)
    return in_maps, idxs


def combine(outs, idxs, inputs_weight, top_idx, b2):
    """Host-side combine: weighted scatter-add back to token positions."""
    vals = []
    for e in range(E):
        v = outs[e]
        if np.any(b2[e]):
            w_e = inputs_weight[idxs[e], e].astype(np.float32)
            v = v + w_e[:, None] * b2[e][None, :].astype(np.float32)
        vals.append(v)
    vals = np.concatenate(vals, axis=0)          # [E*C, D]
    idx_all = np.concatenate(idxs, axis=0)       # [E*C]

    order = np.argsort(idx_all, kind="stable")
    si = idx_all[order]
    sv = vals[order]
    starts = np.flatnonzero(np.r_[True, si[1:] != si[:-1]])
    sums = np.add.reduceat(sv, starts, axis=0)
    res = np.zeros((T, D), dtype=np.float32)
    res[si[starts]] = sums
    return res


def kernel(inputs, inputs_weight, top_idx, W1, b1, W2, b2):
    inputs = np.asarray(inputs, dtype=np.float32)
    inputs_weight = np.asarray(inputs_weight, dtype=np.float32)
    top_idx = np.asarray(top_idx)
    W1 = np.asarray(W1, dtype=np.float32)
    b1 = np.asarray(b1, dtype=np.float32)
    W2 = np.asarray(W2, dtype=np.float32)
    b2 = np.asarray(b2, dtype=np.float32)

    nc = get_nc()
    in_maps, idxs = make_in_maps(
        inputs, inputs_weight, top_idx, W1, b1, W2, b2)
    r = run_bass_kernel_spmd(nc, in_maps, list(range(E)))
    outs = [r.results[e]["out"] for e in range(E)]
    return combine(outs, idxs, inputs_weight, top_idx, b2)


# revision 8
# speedup vs baseline: 4.9671x; 1.2322x over previous
"""MoE expert-FFN (nn_Experts) Trainium2 kernel.

Expert-parallel: one expert per NeuronCore (E = 8 = n_cores).
Host does the token gather (dispatch) and weighted scatter-add (combine);
each core runs the fused FFN for its expert:

    hT = gelu_tanh(W1^T @ tokT + b1)        # [F, C] on chip, f32r matmuls
    out = (hT^T @ W2) * w[:, None]          # [C, D], combine weight fused
                                            # into the PSUM eviction

Layouts are pre-packed on host so every DMA is contiguous-row strided:
    tokT [P, D/P, C]   (d = kc*P + p)
    W1   [P, D/P, F]   (d = kc*P + p)
    W2   [P, F/P, D]   (f = kc*P + p)
    b1t  [P, F/P]      (f = fb*P + p)
    wv   [P, C/CB, CB/P]  (c = cb*CB + m*P + p)
"""
import numpy as np

import concourse.bacc as bacc
import concourse.tile as tile
from concourse import mybir
from concourse.bass_utils import run_bass_kernel_spmd

P = 128
T, D, F, E, C = 8192, 2048, 8192, 8, 2048
CB = 512          # capacity block held resident as hT [F, CB]
NOUT = 512        # output free-dim tile (one PSUM bank of fp32)

f32 = mybir.dt.float32
f32r = mybir.dt.float32r
AF = mybir.ActivationFunctionType


def build_nc(d=None, f=None, c=None, cb=None, nout=None, fake_weights=False):
    d = D if d is None else d
    f = F if f is None else f
    c = C if c is None else c
    cb = CB if cb is None else cb
    nout = NOUT if nout is None else nout
    KD = d // P       # mm1 contraction chunks
    KF = f // P       # mm2 contraction chunks
    FB = f // P       # mm1 output partition groups
    MB = cb // P      # output row subtiles per c-block
    NB = d // nout    # output col tiles
    NCB = c // cb     # c blocks

    _w1_cache = {}
    _w2_cache = {}
    nc = bacc.Bacc()
    G = 2             # W2 kc chunks batched per DMA
    tokT = nc.declare_dram_parameter("tokT", [NCB, P, KD, cb], f32r,
                                     isOutput=False)
    w1 = nc.declare_dram_parameter("w1", [FB, P, KD, P], f32r, isOutput=False)
    w2 = nc.declare_dram_parameter("w2", [NB, KF // G, P, G, nout], f32r,
                                    isOutput=False)
    b1t = nc.declare_dram_parameter("b1t", [P, FB], f32, isOutput=False)
    wv = nc.declare_dram_parameter("wv", [P, NCB, MB], f32, isOutput=False)
    out = nc.declare_dram_parameter("out", [c, d], f32, isOutput=True)

    with tile.TileContext(nc) as tc:
        with tc.tile_pool(name="const", bufs=1) as const, \
             tc.tile_pool(name="tokp", bufs=1) as tokp, \
             tc.tile_pool(name="hp", bufs=1) as hp, \
             tc.tile_pool(name="w1p", bufs=2) as w1p, \
             tc.tile_pool(name="w2p", bufs=3) as w2p, \
             tc.tile_pool(name="ostp", bufs=2) as ostp, \
             tc.tile_pool(name="php", bufs=3, space="PSUM") as php, \
             tc.tile_pool(name="pop", bufs=1, space="PSUM") as pop:
            b1s = const.tile([P, FB], f32)
            nc.sync.dma_start(b1s[:], b1t[:])
            wvs = const.tile([P, NCB, MB], f32)
            nc.sync.dma_start(wvs[:], wv[:])

            for cbi in range(NCB):
                tok_c = tokp.tile([P, KD, cb], f32r, tag="tok")
                nc.sync.dma_start(tok_c[:], tokT[cbi])
                hT = hp.tile([P, KF, cb], f32r, tag="hT")

                # mm1: hT[f, :] = gelu(W1^T @ tokT + b1), one 128-row f group
                # per PSUM accumulation
                for fb in range(FB):
                    if fake_weights:
                        if cbi == 0 and fb == 0:
                            w1t = w1p.tile([P, KD, P], f32r, tag="w1t")
                            nc.sync.dma_start(w1t[:], w1[0])
                            _w1_cache[0] = w1t
                        w1t = _w1_cache[0]
                    else:
                        w1t = w1p.tile([P, KD, P], f32r, tag="w1t")
                        nc.sync.dma_start(w1t[:], w1[fb])
                    ph = php.tile([P, cb], f32, tag="ph")
                    for kc in range(KD):
                        nc.tensor.matmul(ph[:], w1t[:, kc, :], tok_c[:, kc, :],
                                         start=(kc == 0), stop=(kc == KD - 1))
                    # fast DVE drain of PSUM, then gelu in place on ACT off
                    # the PE critical path
                    nc.vector.tensor_copy(hT[:, fb, :], ph[:])
                    nc.scalar.activation(hT[:, fb, :], hT[:, fb, :],
                                         AF.Gelu_apprx_tanh,
                                         bias=b1s[:, fb:fb + 1])

                # mm2: out[c, :] = (hT^T @ W2) * w, accumulated over all of F
                for nb in range(NB):
                    pos = [pop.tile([P, nout], f32, tag=f"po{m}", name=f"po{m}")
                           for m in range(MB)]
                    for kg in range(KF // G):
                        if fake_weights:
                            if cbi == 0 and nb == 0 and kg == 0:
                                w2t = w2p.tile([P, G, nout], f32r, tag="w2t")
                                nc.sync.dma_start(w2t[:], w2[0, 0])
                                _w2_cache[0] = w2t
                            w2t = _w2_cache[0]
                        else:
                            w2t = w2p.tile([P, G, nout], f32r, tag="w2t")
                            nc.sync.dma_start(w2t[:], w2[nb, kg])
                        for g in range(G):
                            kc = kg * G + g
                            for m in range(MB):
                                nc.tensor.matmul(pos[m][:],
                                                 hT[:, kc, m * P:(m + 1) * P],
                                                 w2t[:, g, :],
                                                 start=(kc == 0),
                                                 stop=(kc == KF - 1))
                    for m in range(MB):
                        ost = ostp.tile([P, nout], f32, tag="ost")
                        nc.vector.tensor_tensor(
                            ost[:], pos[m][:],
                            wvs[:, cbi, m:m + 1].to_broadcast((P, nout)),
                            mybir.AluOpType.mult)
                        r0 = cbi * cb + m * P
                        nc.sync.dma_start(
                            out[r0:r0 + P, nb * nout:(nb + 1) * nout], ost[:])
    nc.compile()
    return nc


def pack_core(inputs, inputs_weight, top_idx, W1, b1, e,
              d=None, f=None, c=None, cb=None):
    d = D if d is None else d
    f = F if f is None else f
    c = C if c is None else c
    cb = CB if cb is None else cb
    """Host-side dispatch: gather + relayout for expert e."""
    KD = d // P
    FB = f // P
    MB = cb // P
    NCB = c // cb
    idx = np.asarray(top_idx[:, e])
    tok = np.ascontiguousarray(inputs[idx])                      # [c, d]
    # tokT[cbi, p, kc, j] = tok[cbi*cb + j, kc*P + p]
    tokT = tok.T.reshape(KD, P, NCB, cb).transpose(2, 1, 0, 3)
    # w1m[fb, p, kc, j] = W1[kc*P + p, fb*P + j]
    w1m = W1[e].reshape(KD, P, FB, P).transpose(2, 1, 0, 3)
    b1m = np.ascontiguousarray(b1[e]).reshape(FB, P).T
    wvm = np.ascontiguousarray(inputs_weight[idx, e]) \
        .reshape(NCB, MB, P).transpose(2, 0, 1)
    return idx, tok, tokT, w1m, b1m, wvm


_NC_CACHE = {}


def get_nc():
    key = (D, F, C, CB, NOUT)
    if key not in _NC_CACHE:
        _NC_CACHE[key] = build_nc()
    return _NC_CACHE[key]


def make_in_maps(inputs, inputs_weight, top_idx, W1, b1, W2, b2):
    KF = F // P
    G = 2
    NB = D // NOUT
    in_maps = []
    idxs = []
    for e in range(E):
        idx, _tok, tokT, w1m, b1m, wvm = pack_core(
            inputs, inputs_weight, top_idx, W1, b1, e)
        # w2m[nb, kg, p, g, j] = W2[(kg*G+g)*P + p, nb*NOUT + j]
        w2m = W2[e].reshape(KF // G, G, P, NB, NOUT).transpose(3, 0, 2, 1, 4)
        in_maps.append({
            "tokT": np.ascontiguousarray(tokT, dtype=np.float32),
            "w1": np.ascontiguousarray(w1m, dtype=np.float32),
            "w2": np.ascontiguousarray(w2m, dtype=np.float32),
            "b1t": np.ascontiguousarray(b1m, dtype=np.float32),
            "wv": np.ascontiguousarray(wvm, dtype=np.float32),
        })
        idxs.append(idx)
    return in_maps, idxs


def combine(outs, idxs, inputs_weight, top_idx, b2):
    """Host-side combine: weighted scatter-add back to token positions."""
    vals = []
    for e in range(E):
        v = outs[e]
        if np.any(b2[e]):
            w_e = inputs_weight[idxs[e], e].astype(np.float32)
            v = v + w_e[:, None] * b2[e][None, :].astype(np.float32)
        vals.append(v)
    vals = np.concatenate(vals, axis=0)          # [E*C, D]
    idx_all = np.concatenate(idxs, axis=0)       # [E*C]

    order = np.argsort(idx_all, kind="stable")
    si = idx_all[order]
    sv = vals[order]
    starts = np.flatnonzero(np.r_[True, si[1:] != si[:-1]])
    sums = np.add.reduceat(sv, starts, axis=0)
    res = np.zeros((T, D), dtype=np.float32)
    res[si[starts]] = sums
    return res


def kernel(inputs, inputs_weight, top_idx, W1, b1, W2, b2):
    inputs = np.asarray(inputs, dtype=np.float32)
    inputs_weight = np.asarray(inputs_weight, dtype=np.float32)
    top_idx = np.asarray(top_idx)
    W1 = np.asarray(W1, dtype=np.float32)
    b1 = np.asarray(b1, dtype=np.float32)
    W2 = np.asarray(W2, dtype=np.float32)
    b2 = np.asarray(b2, dtype=np.float32)

    nc = get_nc()
    in_maps, idxs = make_in_maps(
        inputs, inputs_weight, top_idx, W1, b1, W2, b2)
    r = run_bass_kernel_spmd(nc, in_maps, list(range(E)))
    outs = [r.results[e]["out"] for e in range(E)]
    return combine(outs, idxs, inputs_weight, top_idx, b2)


# revision 11
# speedup vs baseline: 5.0301x; 1.0127x over previous
"""MoE expert-FFN (nn_Experts) Trainium2 kernel.

Expert-parallel: one expert per NeuronCore (E = 8 = n_cores).
Host does the token gather (dispatch) and weighted scatter-add (combine);
each core runs the fused FFN for its expert:

    hT = gelu_tanh(W1^T @ tokT + b1)        # [F, C] on chip, f32r matmuls
    out = (hT^T @ W2) * w[:, None]          # [C, D], combine weight fused
                                            # into the PSUM eviction

Layouts are pre-packed on host so every DMA is contiguous-row strided:
    tokT [P, D/P, C]   (d = kc*P + p)
    W1   [P, D/P, F]   (d = kc*P + p)
    W2   [P, F/P, D]   (f = kc*P + p)
    b1t  [P, F/P]      (f = fb*P + p)
    wv   [P, C/CB, CB/P]  (c = cb*CB + m*P + p)
"""
import numpy as np

import concourse.bacc as bacc
import concourse.tile as tile
from concourse import mybir
from concourse.bass_utils import run_bass_kernel_spmd

P = 128
T, D, F, E, C = 8192, 2048, 8192, 8, 2048
CB = 512          # capacity block held resident as hT [F, CB]
NOUT = 512        # output free-dim tile (one PSUM bank of fp32)

f32 = mybir.dt.float32
f32r = mybir.dt.float32r
AF = mybir.ActivationFunctionType


def build_nc(d=None, f=None, c=None, cb=None, nout=None):
    d = D if d is None else d
    f = F if f is None else f
    c = C if c is None else c
    cb = CB if cb is None else cb
    nout = NOUT if nout is None else nout
    KD = d // P       # mm1 contraction chunks
    KF = f // P       # mm2 contraction chunks
    FB = f // P       # mm1 output partition groups
    MB = cb // P      # output row subtiles per c-block
    NB = d // nout    # output col tiles
    NCB = c // cb     # c blocks

    nc = bacc.Bacc()
    G = 2             # W2 kc chunks batched per DMA
    tokT = nc.declare_dram_parameter("tokT", [NCB, P, KD, cb], f32r,
                                     isOutput=False)
    w1 = nc.declare_dram_parameter("w1", [FB, P, KD, P], f32r, isOutput=False)
    w2 = nc.declare_dram_parameter("w2", [NB, KF // G, P, G, nout], f32r,
                                    isOutput=False)
    b1t = nc.declare_dram_parameter("b1t", [P, FB], f32, isOutput=False)
    wv = nc.declare_dram_parameter("wv", [P, NCB, MB], f32, isOutput=False)
    out = nc.declare_dram_parameter("out", [c, d], f32, isOutput=True)

    with tile.TileContext(nc) as tc:
        with tc.tile_pool(name="const", bufs=1) as const, \
             tc.tile_pool(name="tokp", bufs=1) as tokp, \
             tc.tile_pool(name="hp", bufs=1) as hp, \
             tc.tile_pool(name="w1p", bufs=2) as w1p, \
             tc.tile_pool(name="w2p", bufs=3) as w2p, \
             tc.tile_pool(name="ostp", bufs=2) as ostp, \
             tc.tile_pool(name="php", bufs=3, space="PSUM") as php, \
             tc.tile_pool(name="pop", bufs=1, space="PSUM") as pop:
            b1s = const.tile([P, FB], f32)
            nc.sync.dma_start(b1s[:], b1t[:])
            wvs = const.tile([P, NCB, MB], f32)
            nc.sync.dma_start(wvs[:], wv[:])

            for cbi in range(NCB):
                tok_c = tokp.tile([P, KD, cb], f32r, tag="tok")
                nc.sync.dma_start(tok_c[:], tokT[cbi])
                hT = hp.tile([P, KF, cb], f32r, tag="hT")

                # mm1: hT[f, :] = gelu(W1^T @ tokT + b1), one 128-row f group
                # per PSUM accumulation
                for fb in range(FB):
                    w1t = w1p.tile([P, KD, P], f32r, tag="w1t")
                    nc.sync.dma_start(w1t[:], w1[fb])
                    ph = php.tile([P, cb], f32, tag="ph")
                    for kc in range(KD):
                        nc.tensor.matmul(ph[:], w1t[:, kc, :], tok_c[:, kc, :],
                                         start=(kc == 0), stop=(kc == KD - 1))
                    # fast DVE drain of PSUM, then gelu in place on ACT off
                    # the PE critical path
                    nc.vector.tensor_copy(hT[:, fb, :], ph[:])
                    nc.scalar.activation(hT[:, fb, :], hT[:, fb, :],
                                         AF.Gelu_apprx_tanh,
                                         bias=b1s[:, fb:fb + 1])

                # mm2: out[c, :] = (hT^T @ W2) * w, accumulated over all of F
                for nb in range(NB):
                    pos = [pop.tile([P, nout], f32, tag=f"po{m}", name=f"po{m}")
                           for m in range(MB)]
                    for kg in range(KF // G):
                        w2t = w2p.tile([P, G, nout], f32r, tag="w2t")
                        nc.scalar.dma_start(w2t[:], w2[nb, kg])
                        for g in range(G):
                            kc = kg * G + g
                            for m in range(MB):
                                nc.tensor.matmul(pos[m][:],
                                                 hT[:, kc, m * P:(m + 1) * P],
                                                 w2t[:, g, :],
                                                 start=(kc == 0),
                                                 stop=(kc == KF - 1))
                    for m in range(MB):
                        ost = ostp.tile([P, nout], f32, tag="ost")
                        nc.vector.tensor_tensor(
                            ost[:], pos[m][:],
                            wvs[:, cbi, m:m + 1].to_broadcast((P, nout)),
                            mybir.AluOpType.mult)
                        r0 = cbi * cb + m * P
                        nc.sync.dma_start(
                            out[r0:r0 + P, nb * nout:(nb + 1) * nout], ost[:])
    nc.compile()
    return nc


def pack_core(inputs, inputs_weight, top_idx, W1, b1, e,
              d=None, f=None, c=None, cb=None):
    d = D if d is None else d
    f = F if f is None else f
    c = C if c is None else c
    cb = CB if cb is None else cb
    """Host-side dispatch: gather + relayout for expert e."""
    KD = d // P
    FB = f // P
    MB = cb // P
    NCB = c // cb
    idx = np.asarray(top_idx[:, e])
    tok = np.ascontiguousarray(inputs[idx])                      # [c, d]
    # tokT[cbi, p, kc, j] = tok[cbi*cb + j, kc*P + p]
    tokT = tok.T.reshape(KD, P, NCB, cb).transpose(2, 1, 0, 3)
    # w1m[fb, p, kc, j] = W1[kc*P + p, fb*P + j]
    w1m = W1[e].reshape(KD, P, FB, P).transpose(2, 1, 0, 3)
    b1m = np.ascontiguousarray(b1[e]).reshape(FB, P).T
    wvm = np.ascontiguousarray(inputs_weight[idx, e]) \
        .reshape(NCB, MB, P).transpose(2, 0, 1)
    return idx, tok, tokT, w1m, b1m, wvm


_NC_CACHE = {}


def get_nc():
    key = (D, F, C, CB, NOUT)
    if key not in _NC_CACHE:
        _NC_CACHE[key] = build_nc()
    return _NC_CACHE[key]


def make_in_maps(inputs, inputs_weight, top_idx, W1, b1, W2, b2):
    KF = F // P
    G = 2
    NB = D // NOUT
    in_maps = []
    idxs = []
    for e in range(E):
        idx, _tok, tokT, w1m, b1m, wvm = pack_core(
            inputs, inputs_weight, top_idx, W1, b1, e)
        # w2m[nb, kg, p, g, j] = W2[(kg*G+g)*P + p, nb*NOUT + j]
        w2m = W2[e].reshape(KF // G, G, P, NB, NOUT).transpose(3, 0, 2, 1, 4)
        in_maps.append({
            "tokT": np.ascontiguousarray(tokT, dtype=np.float32),
            "w1": np.ascontiguousarray(w1m, dtype=np.float32),
            "w2": np.ascontiguousarray(w2m, dtype=np.float32),
            "b1t": np.ascontiguousarray(b1m, dtype=np.float32),
            "wv": np.ascontiguousarray(wvm, dtype=np.float32),
        })
        idxs.append(idx)
    return in_maps, idxs


def combine(outs, idxs, inputs_weight, top_idx, b2):
    """Host-side combine: weighted scatter-add back to token positions."""
    vals = []
    for e in range(E):
        v = outs[e]
        if np.any(b2[e]):
            w_e = inputs_weight[idxs[e], e].astype(np.float32)
            v = v + w_e[:, None] * b2[e][None, :].astype(np.float32)
        vals.append(v)
    vals = np.concatenate(vals, axis=0)          # [E*C, D]
    idx_all = np.concatenate(idxs, axis=0)       # [E*C]

    order = np.argsort(idx_all, kind="stable")
    si = idx_all[order]
    sv = vals[order]
    starts = np.flatnonzero(np.r_[True, si[1:] != si[:-1]])
    sums = np.add.reduceat(sv, starts, axis=0)
    res = np.zeros((T, D), dtype=np.float32)
    res[si[starts]] = sums
    return res


def kernel(inputs, inputs_weight, top_idx, W1, b1, W2, b2):
    inputs = np.asarray(inputs, dtype=np.float32)
    inputs_weight = np.asarray(inputs_weight, dtype=np.float32)
    top_idx = np.asarray(top_idx)
    W1 = np.asarray(W1, dtype=np.float32)
    b1 = np.asarray(b1, dtype=np.float32)
    W2 = np.asarray(W2, dtype=np.float32)
    b2 = np.asarray(b2, dtype=np.float32)

    nc = get_nc()
    in_maps, idxs = make_in_maps(
        inputs, inputs_weight, top_idx, W1, b1, W2, b2)
    try:
        r = run_bass_kernel_spmd(nc, in_maps, list(range(E)))
    except Exception:
        # transient NRT/device hiccups happen; one retry is usually enough
        import time as _time
        _time.sleep(5)
        r = run_bass_kernel_spmd(nc, in_maps, list(range(E)))
    outs = [r.results[e]["out"] for e in range(E)]
    return combine(outs, idxs, inputs_weight, top_idx, b2)


# revision 14
# speedup vs baseline: 5.4282x; 1.0791x over previous
"""MoE expert-FFN (nn_Experts) Trainium2 kernel.

Expert-parallel: one expert per NeuronCore (E = 8 = n_cores).
Host does the token gather (dispatch) and weighted scatter-add (combine);
each core runs the fused FFN for its expert:

    hT = gelu_tanh(W1^T @ tokT + b1)        # [F, C] on chip, f32r matmuls
    out = (hT^T @ W2) * w[:, None]          # [C, D], combine weight fused
                                            # into the PSUM eviction

Layouts are pre-packed on host so every DMA is contiguous-row strided:
    tokT [P, D/P, C]   (d = kc*P + p)
    W1   [P, D/P, F]   (d = kc*P + p)
    W2   [P, F/P, D]   (f = kc*P + p)
    b1t  [P, F/P]      (f = fb*P + p)
    wv   [P, C/CB, CB/P]  (c = cb*CB + m*P + p)
"""
import numpy as np

import concourse.bacc as bacc
import concourse.tile as tile
from concourse import mybir
from concourse.bass_utils import run_bass_kernel_spmd

P = 128
T, D, F, E, C = 8192, 2048, 8192, 8, 2048
CB = 512          # capacity block held resident as hT [F, CB]
NOUT = 512        # output free-dim tile (one PSUM bank of fp32)

f32 = mybir.dt.float32
f32r = mybir.dt.float32r
AF = mybir.ActivationFunctionType


def build_nc(d=None, f=None, c=None, cb=None, nout=None):
    d = D if d is None else d
    f = F if f is None else f
    c = C if c is None else c
    cb = CB if cb is None else cb
    nout = NOUT if nout is None else nout
    KD = d // P       # mm1 contraction chunks
    KF = f // P       # mm2 contraction chunks
    FB = f // P       # mm1 output partition groups
    MB = cb // P      # output row subtiles per c-block
    NB = d // nout    # output col tiles
    NCB = c // cb     # c blocks

    nc = bacc.Bacc()
    tokT = nc.declare_dram_parameter("tokT", [NCB, P, KD, cb], f32r,
                                     isOutput=False)
    w1 = nc.declare_dram_parameter("w1", [FB, P, KD, P], f32r, isOutput=False)
    w2 = nc.declare_dram_parameter("w2", [NB, KF, P, nout], f32r,
                                    isOutput=False)
    b1t = nc.declare_dram_parameter("b1t", [P, FB], f32, isOutput=False)
    wv = nc.declare_dram_parameter("wv", [P, NCB, MB], f32, isOutput=False)
    out = nc.declare_dram_parameter("out", [c, d], f32, isOutput=True)

    with tile.TileContext(nc) as tc:
        with tc.tile_pool(name="const", bufs=1) as const, \
             tc.tile_pool(name="tokp", bufs=1) as tokp, \
             tc.tile_pool(name="hp", bufs=1) as hp, \
             tc.tile_pool(name="w1p", bufs=2) as w1p, \
             tc.tile_pool(name="w2p", bufs=6) as w2p, \
             tc.tile_pool(name="ostp", bufs=2) as ostp, \
             tc.tile_pool(name="php", bufs=3, space="PSUM") as php, \
             tc.tile_pool(name="pop", bufs=1, space="PSUM") as pop:
            b1s = const.tile([P, FB], f32)
            nc.sync.dma_start(b1s[:], b1t[:])
            wvs = const.tile([P, NCB, MB], f32)
            nc.sync.dma_start(wvs[:], wv[:])

            for cbi in range(NCB):
                tok_c = tokp.tile([P, KD, cb], f32r, tag="tok")
                for kq in range(KD):
                    nc.sync.dma_start(tok_c[:, kq, :], tokT[cbi, :, kq, :])
                hT = hp.tile([P, KF, cb], f32r, tag="hT")

                # mm1: hT[f, :] = gelu(W1^T @ tokT + b1), one 128-row f group
                # per PSUM accumulation
                for fb in range(FB):
                    w1t = w1p.tile([P, KD, P], f32r, tag="w1t")
                    for kq in range(0, KD, 4):
                        nc.sync.dma_start(w1t[:, kq:kq + 4, :],
                                          w1[fb, :, kq:kq + 4, :])
                    ph = php.tile([P, cb], f32, tag="ph")
                    for kc in range(KD):
                        nc.tensor.matmul(ph[:], w1t[:, kc, :], tok_c[:, kc, :],
                                         start=(kc == 0), stop=(kc == KD - 1))
                    # fast DVE drain of PSUM, then gelu in place on ACT off
                    # the PE critical path
                    nc.vector.tensor_copy(hT[:, fb, :], ph[:])
                    nc.scalar.activation(hT[:, fb, :], hT[:, fb, :],
                                         AF.Gelu_apprx_tanh,
                                         bias=b1s[:, fb:fb + 1])

                # mm2: out[c, :] = (hT^T @ W2) * w, accumulated over all of F
                for nb in range(NB):
                    pos = [pop.tile([P, nout], f32, tag=f"po{m}", name=f"po{m}")
                           for m in range(MB)]
                    for kc in range(KF):
                        w2t = w2p.tile([P, nout], f32r, tag="w2t")
                        nc.sync.dma_start(w2t[:], w2[nb, kc])
                        for m in range(MB):
                            nc.tensor.matmul(pos[m][:],
                                             hT[:, kc, m * P:(m + 1) * P],
                                             w2t[:],
                                             start=(kc == 0),
                                             stop=(kc == KF - 1))
                    for m in range(MB):
                        ost = ostp.tile([P, nout], f32, tag="ost")
                        nc.vector.tensor_tensor(
                            ost[:], pos[m][:],
                            wvs[:, cbi, m:m + 1].to_broadcast((P, nout)),
                            mybir.AluOpType.mult)
                        r0 = cbi * cb + m * P
                        nc.sync.dma_start(
                            out[r0:r0 + P, nb * nout:(nb + 1) * nout], ost[:])
    nc.compile()
    return nc


def pack_core(inputs, inputs_weight, top_idx, W1, b1, e,
              d=None, f=None, c=None, cb=None):
    d = D if d is None else d
    f = F if f is None else f
    c = C if c is None else c
    cb = CB if cb is None else cb
    """Host-side dispatch: gather + relayout for expert e."""
    KD = d // P
    FB = f // P
    MB = cb // P
    NCB = c // cb
    idx = np.asarray(top_idx[:, e])
    tok = np.ascontiguousarray(inputs[idx])                      # [c, d]
    # tokT[cbi, p, kc, j] = tok[cbi*cb + j, kc*P + p]
    tokT = tok.T.reshape(KD, P, NCB, cb).transpose(2, 1, 0, 3)
    # w1m[fb, p, kc, j] = W1[kc*P + p, fb*P + j]
    w1m = W1[e].reshape(KD, P, FB, P).transpose(2, 1, 0, 3)
    b1m = np.ascontiguousarray(b1[e]).reshape(FB, P).T
    wvm = np.ascontiguousarray(inputs_weight[idx, e]) \
        .reshape(NCB, MB, P).transpose(2, 0, 1)
    return idx, tok, tokT, w1m, b1m, wvm


_NC_CACHE = {}


def get_nc():
    key = (D, F, C, CB, NOUT)
    if key not in _NC_CACHE:
        _NC_CACHE[key] = build_nc()
    return _NC_CACHE[key]


def make_in_maps(inputs, inputs_weight, top_idx, W1, b1, W2, b2):
    KF = F // P
    NB = D // NOUT
    in_maps = []
    idxs = []
    for e in range(E):
        idx, _tok, tokT, w1m, b1m, wvm = pack_core(
            inputs, inputs_weight, top_idx, W1, b1, e)
        # w2m[nb, kc, p, j] = W2[kc*P + p, nb*NOUT + j]
        w2m = W2[e].reshape(KF, P, NB, NOUT).transpose(2, 0, 1, 3)
        in_maps.append({
            "tokT": np.ascontiguousarray(tokT, dtype=np.float32),
            "w1": np.ascontiguousarray(w1m, dtype=np.float32),
            "w2": np.ascontiguousarray(w2m, dtype=np.float32),
            "b1t": np.ascontiguousarray(b1m, dtype=np.float32),
            "wv": np.ascontiguousarray(wvm, dtype=np.float32),
        })
        idxs.append(idx)
    return in_maps, idxs


def combine(outs, idxs, inputs_weight, top_idx, b2):
    """Host-side combine: weighted scatter-add back to token positions."""
    vals = []
    for e in range(E):
        v = outs[e]
        if np.any(b2[e]):
            w_e = inputs_weight[idxs[e], e].astype(np.float32)
            v = v + w_e[:, None] * b2[e][None, :].astype(np.float32)
        vals.append(v)
    vals = np.concatenate(vals, axis=0)          # [E*C, D]
    idx_all = np.concatenate(idxs, axis=0)       # [E*C]

    order = np.argsort(idx_all, kind="stable")
    si = idx_all[order]
    sv = vals[order]
    starts = np.flatnonzero(np.r_[True, si[1:] != si[:-1]])
    sums = np.add.reduceat(sv, starts, axis=0)
    res = np.zeros((T, D), dtype=np.float32)
    res[si[starts]] = sums
    return res


def kernel(inputs, inputs_weight, top_idx, W1, b1, W2, b2):
    inputs = np.asarray(inputs, dtype=np.float32)
    inputs_weight = np.asarray(inputs_weight, dtype=np.float32)
    top_idx = np.asarray(top_idx)
    W1 = np.asarray(W1, dtype=np.float32)
    b1 = np.asarray(b1, dtype=np.float32)
    W2 = np.asarray(W2, dtype=np.float32)
    b2 = np.asarray(b2, dtype=np.float32)

    nc = get_nc()
    in_maps, idxs = make_in_maps(
        inputs, inputs_weight, top_idx, W1, b1, W2, b2)
    try:
        r = run_bass_kernel_spmd(nc, in_maps, list(range(E)))
    except Exception:
        # transient NRT/device hiccups happen; one retry is usually enough
        import time as _time
        _time.sleep(5)
        r = run_bass_kernel_spmd(nc, in_maps, list(range(E)))
    outs = [r.results[e]["out"] for e in range(E)]
    return combine(outs, idxs, inputs_weight, top_idx, b2)
